# revision 28
# baseline (speedup 1.0000x reference)
"""Grouped GEMM (MoE routing) Trainium2 kernel.

Full inputs in, full output out. Tensor-parallel shard of the output N
dimension across 8 NeuronCores (each core: all 8192 tokens x a 512-column
slice of N). Matmul orientation: weights stationary ([128 K, 128 N] tiles
loaded into the PE array), tokens as the moving dimension -- token segments
need no 128-padding, so the tensor engine streams exactly T*K*NS MACs.

Mixed precision: the leading NF8=2 k-blocks (k=0..255) run as one fp8-e4m3
DoubleRow matmul per (N-slice, piece) at 2x bf16 throughput (operands
pre-scaled a/16 and b*16 to center e4m3's range); the remaining 14 k-blocks
run in bf16. Measured rel-fro error 1.36e-2 vs the 2e-2 budget.

Input DMA triggers are emitted just-in-time (interleaved with compute) on
the Sync engine in first-use order with kb-granule tiles, so matmuls gate
on quarter-tiles; output strips drain on the Scalar engine so the in-order
trigger queues never head-of-line block each other. Early supersegments
run kb-outer (4 N-slices inside the kb loop) to spread pipeline-fill DMA
demand. A few throwaway warmup matmuls keep the PE clock ramping while the
first tiles land. Outputs drain as bf16 [N-slice, 1024-token window]
strips, the final window in halves fanned across engines. A post-pass
drops InstLdweights that reload the PE with identical weights. The
host-known segment schedule is baked into the instruction stream; the host
transposes/reassembles the output.
"""

import os
import sys
from contextlib import ExitStack

import numpy as np

for _p in ("/opt/trn_rl_repo", "/root/.axon_site/_ro/trn_rl_repo"):
    if os.path.isdir(_p) and _p not in sys.path:
        sys.path.insert(0, _p)

import concourse.bass as bass  # noqa: E402,F401
import concourse.tile as tile  # noqa: E402
from concourse import bacc, mybir  # noqa: E402

E, T, K, N = 8, 8192, 2048, 4096
NCORES = 8
NS = N // NCORES  # output columns per core (512)
P = 128
KB = K // P  # contraction blocks (16)
BT = 512  # tokens per activation block (moving-dim granularity)
NBLK = T // BT  # 16 activation blocks
WIN = 1024  # output window (tokens) per drain strip
NSL = NS // P  # 4 stationary N-slices per core
PREFETCH_TOK = 800  # issue input DMA triggers this many tokens ahead
KBOUTER_TOK = 2400  # supersegs starting below this use kb-outer emission
NF8 = int(os.environ.get("KERNEL_NF8", "2"))  # leading kb-blocks done in fp8 DoubleRow (0 or 2)
KB0 = NF8  # first bf16 kb-block
LAST_RESULT = None  # BassKernelResults of the most recent run (for test.py)


def _dtypes():
    kind = os.environ.get("KERNEL_DTYPE", "bf16")
    if kind == "bf16":
        import ml_dtypes

        return mybir.dt.bfloat16, ml_dtypes.bfloat16
    if kind == "f32":
        return mybir.dt.float32, np.float32
    return mybir.dt.float32r, np.float32


def _schedule(seg):
    """Token-exact schedule. Returns (ss_list, window_last).

    ss_list: supersegments (slot, [(block, o0, o1, t0), ...]) with <=4
    pieces each; a piece is a segment's token run within one 512 block.
    window_last[w]: index in ss_list of the last supersegment touching
    output window w.
    """
    ss_list = []
    for s in range(E):
        lo, hi = int(seg[s]), int(seg[s + 1])
        if hi <= lo:
            continue
        pieces = []
        t = lo
        while t < hi:
            b = t // BT
            t1 = min((b + 1) * BT, hi)
            pieces.append((b, t - b * BT, t1 - b * BT, t))
            t = t1
        # early supersegs run kb-outer (all 4 N-slices inside the kb loop,
        # 8 PSUM banks) which spreads their DMA demand over 4x the time --
        # crucial while the pipeline fills; those need <=2 pieces
        gsz = 2 if lo < KBOUTER_TOK else 4
        groups = [pieces[i : i + gsz] for i in range(0, len(pieces), gsz)]
        if len(groups) >= 2 and len(groups[-2]) > 1:
            # avoid a tiny trailing group (stationary-load-bound matmuls)
            if sum(p[2] - p[1] for p in groups[-1]) < 256:
                groups[-1].insert(0, groups[-2].pop())
        for g in groups:
            ss_list.append((s, g))
    window_last = {}
    for i, (_, pieces) in enumerate(ss_list):
        for b, _, _, t0 in pieces:
            window_last[t0 // WIN] = i
    return ss_list, window_last


def _dedup_ldweights(nc):
    """Drop InstLdweights that reload the PE array with the exact weights the
    previous InstLdweights on the same engine already loaded (only matmuls in
    between). Loaded weights persist in the PE array across matmuls, so the
    reload is pure overhead; novel semaphore waits are carried onto the next
    kept instruction.
    """
    from concourse import mybir

    for f in nc.m.functions:
        for bb in f.blocks:
            insts = bb.instructions
            new = []
            last_sig = None
            satisfied = {}  # sem id -> max value this engine already waited for
            pending = []  # waits carried from dropped ldweights

            def note_waits(si):
                if si is None:
                    return
                for w in si.on_wait:
                    if w.sync_type == "semaphore" and w.wait_mode == "sem-ge-imm":
                        if satisfied.get(w.id, -1) < w.wait_value:
                            satisfied[w.id] = w.wait_value

            def novel(waits):
                out = []
                for w in waits:
                    if (
                        w.sync_type == "semaphore"
                        and w.wait_mode == "sem-ge-imm"
                        and satisfied.get(w.id, -1) >= w.wait_value
                    ):
                        continue
                    out.append(w)
                return out

            for inst in insts:
                nm = type(inst).__name__
                if nm == "InstLdweights":
                    ap = inst.ins[0]
                    sig = (
                        ap.memref,
                        ap.offset,
                        str(ap.ap),
                        str(ap.dtype),
                        str(inst.is_transpose),
                    )
                    si = inst.sync_info
                    ups = list(si.on_update) if si else []
                    if sig == last_sig and not ups:
                        pending.extend(novel(list(si.on_wait) if si else []))
                        continue
                    last_sig = sig
                elif nm != "InstMatmult":
                    last_sig = None
                if pending:
                    si = inst.sync_info
                    waits = pending + (list(si.on_wait) if si else [])
                    ups = list(si.on_update) if si else []
                    inst.sync_info = mybir.SyncInfo(on_wait=waits, on_update=ups)
                    pending = []
                note_waits(inst.sync_info)
                new.append(inst)
            assert not pending
            bb.instructions[:] = new


def _build(seg_vals, mm_dt):
    nc = bacc.Bacc(
        "TRN2",
        target_bir_lowering=False,
        debug=False,
        enable_asserts=False,
        num_devices=NCORES,
    )
    f32 = mybir.dt.float32
    # SBUF-native HBM layouts (contiguous runs per partition line):
    # at[b, p, kb, t] = a[b*512 + t, kb*128 + p]
    # bt[s, p, kb, n] = b[widx[s], n_off + n, kb*128 + p]
    # out[nsl, p, t]  = c[t, n_off + nsl*128 + p]
    at = nc.dram_tensor("at", [NBLK, P, KB, BT], mm_dt, kind="ExternalInput").ap()
    bt = nc.dram_tensor("bt", [E, P, KB, NS], mm_dt, kind="ExternalInput").ap()
    out = nc.dram_tensor("out", [NSL, P, T], mm_dt, kind="ExternalOutput").ap()
    if NF8:
        f8 = mybir.dt.float8e4
        a8 = nc.dram_tensor("a8", [NBLK, P, NF8, BT], f8, kind="ExternalInput").ap()
        w8 = nc.dram_tensor("w8", [E, P, NF8, NS], f8, kind="ExternalInput").ap()

    ss_list, window_last = _schedule(seg_vals)
    ss_start = [pieces[0][3] for _, pieces in ss_list]
    first_slot = ss_list[0][0]
    first_blocks = {b for b, _, _, _ in ss_list[0][1]}

    with tile.TileContext(nc) as tc, ExitStack() as ctx:
        # all input tiles are kb-granule slices so matmuls gate on partial
        # tiles; extra-fine granules for the very first weight slot + blocks
        ffwpool = ctx.enter_context(tc.tile_pool(name="ffw", bufs=2))
        ffapool = ctx.enter_context(tc.tile_pool(name="ffa", bufs=4))
        w0pool = ctx.enter_context(tc.tile_pool(name="w0", bufs=7))
        a0pool = ctx.enter_context(tc.tile_pool(name="a0", bufs=14))
        wpool = ctx.enter_context(tc.tile_pool(name="w", bufs=10))
        apool = ctx.enter_context(tc.tile_pool(name="a", bufs=16 if NF8 else 20))
        pspool = ctx.enter_context(tc.tile_pool(name="ps", bufs=8, space="PSUM"))
        opool = ctx.enter_context(tc.tile_pool(name="o", bufs=10))
        if NF8:
            a8pool = ctx.enter_context(tc.tile_pool(name="a8", bufs=20))
            w8pool = ctx.enter_context(tc.tile_pool(name="w8", bufs=10))

        # ---- input DMA events, ordered by first use -----------------------
        first_use_a = {}
        first_use_w = {}
        for s, pieces in ss_list:
            t_start = pieces[0][3]
            first_use_w.setdefault(s, t_start)
            for b, _, _, _ in pieces:
                first_use_a.setdefault(b, t_start)
        nbf = KB - KB0  # bf16 kb-blocks
        fine_plan = (1, 1) + (2,) * ((nbf - 2) // 2)
        coarse_plan = ((2,) + (4,) * ((nbf - 2) // 4)) if nbf % 4 else (4,) * (nbf // 4)
        events = []  # (pos, kb_lo, priority, kind, idx, klo, khi)
        for s, pos in first_use_w.items():
            granules = fine_plan if s == first_slot else coarse_plan
            klo = KB0
            for kg in granules:
                events.append((pos, klo, 0, "w", s, klo, klo + kg))
                klo += kg
            if NF8:
                events.append((pos, KB, 0, "w8", s, 0, NF8))
        for b, pos in first_use_a.items():
            granules = fine_plan if b in first_blocks else coarse_plan
            klo = KB0
            for kg in granules:
                events.append((pos, klo, 1, "a", b, klo, klo + kg))
                klo += kg
            if NF8:
                events.append((pos, KB, 1, "a8", b, 0, NF8))
        events.sort(key=lambda e: (e[0], e[1], e[2], e[4]))
        # assign each event the superseg index before which it is emitted:
        # the last superseg starting at-or-before the issue point
        import bisect

        ev_issue = []
        for ev in events:
            issue_tok = ev[0] - PREFETCH_TOK
            ev_issue.append(max(0, bisect.bisect_right(ss_start, issue_tok) - 1))

        a_sub = {b: [None] * KB for b in range(NBLK)}
        w_sub = {s: [None] * KB for s in first_use_w}
        a8_t = {}
        w8_t = {}

        def emit_input_dmas(ss_i):
            while events and ev_issue[0] <= ss_i:
                pos, _, _, kind, idx, klo, khi = events.pop(0)
                ev_issue.pop(0)
                kw = khi - klo
                trig = nc.sync
                if kind == "w8":
                    st = w8pool.tile([P, NF8, NS], f8, tag="w8", name="w8t")
                    trig.dma_start(out=st[:], in_=w8[idx])
                    w8_t[idx] = st
                elif kind == "a8":
                    st = a8pool.tile([P, NF8, BT], f8, tag="a8", name="a8t")
                    trig.dma_start(out=st[:], in_=a8[idx])
                    a8_t[idx] = st
                elif kind == "w":
                    pool = (
                        (ffwpool if kw == 1 else w0pool)
                        if idx == first_slot
                        else wpool
                    )
                    st = pool.tile([P, kw, NS], mm_dt, tag="w", name="wt")
                    trig.dma_start(out=st[:], in_=bt[idx][:, klo:khi, :])
                    for kb in range(klo, khi):
                        w_sub[idx][kb] = (st, kb - klo)
                else:
                    pool = (
                        (ffapool if kw == 1 else a0pool)
                        if idx in first_blocks
                        else apool
                    )
                    st = pool.tile([P, kw, BT], mm_dt, tag="a", name="atile")
                    trig.dma_start(out=st[:], in_=at[idx][:, klo:khi, :])
                    for kb in range(klo, khi):
                        a_sub[idx][kb] = (st, kb - klo)

        # ---- PE clock warmup ---------------------------------------------
        # The tensor engine ramps to full clock only after ~3us of
        # continuous execution. Run throwaway matmuls on (uninitialized)
        # SBUF while the first input DMAs are still in flight so the real
        # stream starts at full speed. No waits -> they fill otherwise-idle
        # engine time.
        n_warm = int(os.environ.get("KERNEL_WARMUP_MM", "3"))
        if n_warm:
            wsrc = apool.tile([P, 4, BT], mm_dt, tag="a", name="warmsrc")
            nc.gpsimd.memset(wsrc[:, 0, :], 0.0)
            wps = pspool.tile([P, BT], f32, tag="ps", name="warmps")
            for _ in range(n_warm):
                nc.tensor.matmul(
                    wps[:, :],
                    lhsT=wsrc[:, 0, 0:P],
                    rhs=wsrc[:, 0, :],
                    start=True,
                    stop=True,
                )

        # ---- compute + drain ----------------------------------------------
        strips = {}  # (w, nsl) -> strip tile

        # last-ss per 512-token half, to drain the final window eagerly
        half_last = {}
        for i, (_, pcs) in enumerate(ss_list):
            for b, _, _, t0 in pcs:
                half_last[t0 // BT] = i
        last_win = max(window_last)

        def copy_piece(nsl, ps, piece):
            b, o0, o1, t0 = piece
            ln = o1 - o0
            w = t0 // WIN
            key = (w, nsl)
            if key not in strips:
                strips[key] = opool.tile([P, WIN], mm_dt, tag="o", name="ostrip")
            c0 = t0 - w * WIN
            nc.scalar.copy(strips[key][:, c0 : c0 + ln], ps[:, :ln])

        def flush(ss_i, nsl, pieces):
            # flush any window (or final-window half) this (ss, nsl) finishes;
            # casts and these triggers share the Scalar queue, so each trigger
            # runs right after the casts it depends on
            for w in sorted({t0 // WIN for _, _, _, t0 in pieces}):
                if w == last_win:
                    eng = (nc.scalar, nc.sync, nc.scalar, nc.sync)[nsl]
                    for h in (2 * w, 2 * w + 1):
                        if half_last.get(h) == ss_i:
                            c0 = h * BT - w * WIN
                            eng.dma_start(
                                out=out[nsl][:, h * BT : (h + 1) * BT],
                                in_=strips[(w, nsl)][:, c0 : c0 + BT],
                            )
                elif window_last[w] == ss_i:
                    nc.scalar.dma_start(
                        out=out[nsl][:, w * WIN : (w + 1) * WIN],
                        in_=strips[(w, nsl)][:],
                    )

        for ss_i, (slot, pieces) in enumerate(ss_list):
            emit_input_dmas(ss_i)
            if pieces[0][3] < KBOUTER_TOK and len(pieces) <= 2:
                ps_all = [
                    [pspool.tile([P, BT], f32, tag="ps", name="ps") for _ in pieces]
                    for _ in range(NSL)
                ]
                for kb in range(KB0, KB):
                    last = kb == KB - 1 and not NF8
                    for nsl in range(NSL):
                        wt, kl = w_sub[slot][kb]
                        lhsT = wt[:, kl, nsl * P : (nsl + 1) * P]
                        for j, piece in enumerate(pieces):
                            b, o0, o1, t0 = piece
                            atile, akl = a_sub[b][kb]
                            nc.tensor.matmul(
                                ps_all[nsl][j][:, : o1 - o0],
                                lhsT=lhsT,
                                rhs=atile[:, akl, o0:o1],
                                start=(kb == KB0),
                                stop=last,
                            )
                            if last:
                                copy_piece(nsl, ps_all[nsl][j], piece)
                        if last:
                            flush(ss_i, nsl, pieces)
                if NF8:
                    for nsl in range(NSL):
                        lhsT8 = w8_t[slot][:, :, nsl * P : (nsl + 1) * P]
                        for j, piece in enumerate(pieces):
                            b, o0, o1, t0 = piece
                            nc.tensor.matmul(
                                ps_all[nsl][j][:, : o1 - o0],
                                lhsT=lhsT8,
                                rhs=a8_t[b][:, :, o0:o1],
                                start=False,
                                stop=True,
                                perf_mode=mybir.MatmulPerfMode.DoubleRow,
                            )
                            copy_piece(nsl, ps_all[nsl][j], piece)
                        flush(ss_i, nsl, pieces)
            else:
                for nsl in range(NSL):
                    ps_tiles = [
                        pspool.tile([P, BT], f32, tag="ps", name="ps") for _ in pieces
                    ]
                    for kb in range(KB0, KB):
                        wt, kl = w_sub[slot][kb]
                        lhsT = wt[:, kl, nsl * P : (nsl + 1) * P]
                        for j, (b, o0, o1, t0) in enumerate(pieces):
                            atile, akl = a_sub[b][kb]
                            nc.tensor.matmul(
                                ps_tiles[j][:, : o1 - o0],
                                lhsT=lhsT,
                                rhs=atile[:, akl, o0:o1],
                                start=(kb == KB0),
                                stop=(kb == KB - 1 and not NF8),
                            )
                    if NF8:
                        lhsT8 = w8_t[slot][:, :, nsl * P : (nsl + 1) * P]
                        for j, (b, o0, o1, t0) in enumerate(pieces):
                            nc.tensor.matmul(
                                ps_tiles[j][:, : o1 - o0],
                                lhsT=lhsT8,
                                rhs=a8_t[b][:, :, o0:o1],
                                start=False,
                                stop=True,
                                perf_mode=mybir.MatmulPerfMode.DoubleRow,
                            )
                    for ps, piece in zip(ps_tiles, pieces):
                        copy_piece(nsl, ps, piece)
                    flush(ss_i, nsl, pieces)

    if os.environ.get("KERNEL_LDW_DEDUP", "1") == "1":
        _dedup_ldweights(nc)
    nc.compile()
    return nc


def kernel(a, b, c, batch_size, weight_column_major, seg_indptr, weight_indices, **_):
    from concourse.bass_utils import run_bass_kernel_spmd

    global LAST_RESULT
    mm_dt, np_dt = _dtypes()

    a = np.asarray(a, dtype=np.float32)
    b = np.asarray(b, dtype=np.float32)
    seg = [int(x) for x in np.asarray(seg_indptr)]
    widx = [int(x) for x in np.asarray(weight_indices)]

    # at[b, p, kb, t] = a[b*512 + t, kb*128 + p]
    aT = np.ascontiguousarray(a.T).astype(np_dt, copy=False)  # [K, T]
    at_tiled = np.ascontiguousarray(
        aT.reshape(KB, P, NBLK, BT).transpose(2, 1, 0, 3)
    )  # [NBLK, P, KB, BT]

    bperm = b[widx]  # [E, N, K] in segment-slot order
    if NF8:
        import ml_dtypes

        np_f8 = ml_dtypes.float8_e4m3fn
        # a8[b, p, i, t] = f8(a[b*512+t, i*128+p] / 16); w8 carries the x16
        a8_tiled = np.ascontiguousarray(
            (aT[: NF8 * P].astype(np.float32) / 16.0)
            .reshape(NF8, P, NBLK, BT)
            .transpose(2, 1, 0, 3)
        ).astype(np_f8)
    in_maps = []
    for cidx in range(NCORES):
        btc = np.swapaxes(bperm[:, cidx * NS : (cidx + 1) * NS, :], 1, 2)  # [E,K,NS]
        bt_tiled = np.ascontiguousarray(
            btc.reshape(E, KB, P, NS).transpose(0, 2, 1, 3)
        ).astype(np_dt, copy=False)  # [E, P, KB, NS]
        im = {"at": at_tiled, "bt": bt_tiled}
        if NF8:
            btc8 = np.swapaxes(
                bperm[:, cidx * NS : (cidx + 1) * NS, : NF8 * P] * 16.0, 1, 2
            )  # [E, NF8*P, NS]
            im["w8"] = np.ascontiguousarray(
                btc8.reshape(E, NF8, P, NS).transpose(0, 2, 1, 3)
            ).astype(np_f8)
            im["a8"] = a8_tiled
        in_maps.append(im)

    nc = _build(seg, mm_dt)
    trace = bool(int(os.environ.get("KERNEL_TRACE", "0")))
    tmpdir = None
    if trace:
        import shutil

        tmpdir = os.environ.get("KERNEL_TRACE_DIR", "/tmp/ntff_out")
        shutil.rmtree(tmpdir, ignore_errors=True)
        os.makedirs(tmpdir, exist_ok=True)
    res = run_bass_kernel_spmd(
        nc,
        in_maps,
        core_ids=list(range(NCORES)),
        trace=trace,
        tmpdir=tmpdir,
    )
    LAST_RESULT = res

    # out[nsl, p, t] per core -> full [T, N] fp32
    full = np.empty((N, T), dtype=np.float32)
    for cidx in range(NCORES):
        oc = np.asarray(res.results[cidx]["out"]).reshape(NS, T)
        full[cidx * NS : (cidx + 1) * NS, :] = oc.astype(np.float32)
    return np.ascontiguousarray(full.T)
